# revision 1
# baseline (speedup 1.0000x reference)
"""Trainium2 Bass kernel for nn_CausalSelfAttention_57861799412149.

Self-contained: takes FULL inputs (as in reference.setup_inputs()), returns the
FULL output. Sharding: sequence-parallel — 8 cores = 2 batches x 4 contiguous
query chunks of 512; each core computes K/V only for its 1536-key window
(query chunk + 1024 lookahead, zero-padded past T) and writes an exact
[512, 1024] slice of the output. No collectives.

v3 notes:
- All big matmuls in bf16 (inputs cast on host); PSUM accumulation in f32.
- Sliding-window masks folded into the S^T matmul accumulation as
  (-60*I) x excl-triangle bf16 matmuls — no post-exp masking ops.
- rmsnorm 1/sqrt via DVE Newton iteration (bit-trick seed, 2 steps), batched
  4 ropes per PSUM tile at 32-aligned partition slots. No ACT Sqrt at all, so
  the single ACT table set (exp/tanh/square/copy) loads once and attention
  never waits on table switches or rope grouping.
- Rope chain spread across engines: ACT copy (PSUM->SBUF) + Square, Pool
  muls with host-preshuffled cos/sin, DVE shuffle/add; per-token scales are
  broadcast across partitions with a bc2-style matmul.
- Softmax has no max-subtraction (|s| <= 8); denominators come free from a
  ones-column appended to V; normalization applied to O^T.
"""
import sys

sys.path.insert(0, "/opt/trn_rl_repo")

import numpy as np
import ml_dtypes

import concourse.bass as bass
import concourse.tile as tile
from concourse import bacc, mybir

B, T, NE = 2, 2048, 1024
NH, NKV, HD = 16, 4, 64
CH = 512            # queries per core
NK = 1536           # key window per core (padded)
TPAD = 3072
EK = NE // 128      # 8 contraction tiles
NJT = NK // 128     # 12 key tiles

f32 = mybir.dt.float32
f32r = mybir.dt.float32r
bf16 = mybir.dt.bfloat16
i32 = mybir.dt.int32
AF = mybir.ActivationFunctionType
OP = mybir.AluOpType
SWAP_MASK = [m for i in range(0, 32, 2) for m in (i + 1, i)]
MAGIC0 = 0x5F3759DF  # rsqrt bit-trick seed magic: y0 = bitcast(MAGIC - (i >> 1))
# head pairs sharing one [128, .] tile: strips (0, 64). Pair (h, h+4) keeps the
# kv-group parity aligned with the kt pair tiles so matmul base partitions match.
PAIRS = [(0, 4), (1, 5), (2, 6), (3, 7), (8, 12), (9, 13), (10, 14), (11, 15)]

_COMPILED = None


def _r(ap):
    return ap.bitcast(f32r)


def build_program(repeat=1, unroll=False):
    nc = bacc.Bacc("TRN2", target_bir_lowering=False, debug=False, num_devices=8)

    def din(name, shape, dt=bf16):
        return nc.dram_tensor(name, shape, dt, kind="ExternalInput").ap()

    xt_d = din("xt", [NE, NK])
    wq_d = din("wqt", [NE, NE])
    wk_d = din("wkt", [NE, NKV * HD])
    wv_d = din("wvt", [NE, NKV * HD])
    wg_d = din("wgt", [32, NKV])
    wp_d = din("wpt", [NE, NE])
    csa_d = din("csa", [128, NK])
    csbs_d = din("csbs", [128, NK])
    ve_d = din("ve", [NK, NKV * HD])
    on4_d = din("ones4", [128, NJT * NKV])
    exlo_d = din("exlo", [128, 128])
    exup_d = din("exup", [128, 128])
    negi_d = din("negi", [128, 128])
    bd_d = din("bdp", [128, 392], f32)
    bc2r_d = din("bc2r", [128, 128], f32)
    out_d = nc.dram_tensor("out", [CH, NE], f32, kind="ExternalOutput").ap()

    ctx_vars = locals()
    with tile.TileContext(nc) as tc:
        if repeat == 1:
            _build(nc, tc, ctx_vars)
        elif unroll:
            for _ in range(repeat):
                _build(nc, tc, ctx_vars)
        else:
            with tc.For_i(0, repeat,
                          hint_engines=(mybir.EngineType.PE,
                                        mybir.EngineType.DVE,
                                        mybir.EngineType.Activation)):
                _build(nc, tc, ctx_vars)

    nc.compile()
    return nc


def _build(nc, tc, d):
    from contextlib import ExitStack

    ctx = ExitStack()
    with ctx:
        # ---------------- persistent pools (live whole kernel) ----------------
        consts = ctx.enter_context(tc.tile_pool(name="consts", bufs=1))
        qtp = ctx.enter_context(tc.tile_pool(name="qtp", bufs=1))
        ktp = ctx.enter_context(tc.tile_pool(name="ktp", bufs=1))
        vxp = ctx.enter_context(tc.tile_pool(name="vxp", bufs=1))
        ytp = ctx.enter_context(tc.tile_pool(name="ytp", bufs=1))
        wqp = ctx.enter_context(tc.tile_pool(name="wqp", bufs=1))
        wpp = ctx.enter_context(tc.tile_pool(name="wpp", bufs=1))
        xa = ctx.enter_context(tc.tile_pool(name="xa", bufs=1))
        vep = ctx.enter_context(tc.tile_pool(name="vep", bufs=12))
        tmpA = ctx.enter_context(tc.tile_pool(name="tmpA", bufs=3))
        rotp = ctx.enter_context(tc.tile_pool(name="rotp", bufs=6))
        rsq = ctx.enter_context(tc.tile_pool(name="rsq", bufs=2))

        bdp = consts.tile([128, 392], f32r, tag="bdp")
        bc2r = consts.tile([128, 128], f32r, tag="bc2r")
        negi = consts.tile([128, 128], bf16, tag="negi")
        exlo = consts.tile([128, 128], bf16, tag="exlo")
        exup = consts.tile([128, 128], bf16, tag="exup")
        on4 = consts.tile([128, NJT * NKV], bf16, tag="on4")
        zP = consts.tile([128, CH], bf16, tag="zP")
        nc.vector.memset(zP[:], 0.0)
        csa = consts.tile([128, NK], bf16, tag="csa")
        csbs = consts.tile([128, NK], bf16, tag="csbs")

        qt = [qtp.tile([128, CH], bf16, tag=f"qt{p}", name=f"qt{p}") for p in range(8)]
        kt = [ktp.tile([128, NK], bf16, tag=f"kt{t}", name=f"kt{t}") for t in range(2)]
        vx = [vxp.tile([128, NKV * (HD + 1)], bf16, tag=f"vx{j}", name=f"vx{j}") for j in range(NJT)]
        yt = [ytp.tile([128, CH], bf16, tag=f"yt{f}", name=f"yt{f}") for f in range(EK)]
        wq = [wqp.tile([128, NE], bf16, tag=f"wq{e}", name=f"wq{e}") for e in range(EK)]
        wp = [wpp.tile([128, NE], bf16, tag=f"wp{e}", name=f"wp{e}") for e in range(EK)]

        wg = xa.tile([32, NKV], bf16, tag="wg")
        nc.sync.dma_start(wg[:], d["wg_d"][:])
        xt = [xa.tile([128, NK], bf16, tag=f"xt{e}", name=f"xt{e}") for e in range(EK)]
        wk = [xa.tile([128, NKV * HD], bf16, tag=f"wk{e}", name=f"wk{e}") for e in range(EK)]
        wv = [xa.tile([128, NKV * HD], bf16, tag=f"wv{e}", name=f"wv{e}") for e in range(EK)]
        for e in range(EK):
            nc.sync.dma_start(xt[e][:, 0:768], d["xt_d"][128 * e:128 * e + 128, 0:768])
            nc.sync.dma_start(wk[e][:], d["wk_d"][128 * e:128 * e + 128, :])
        nc.sync.dma_start(csa[:], d["csa_d"][:])
        nc.sync.dma_start(csbs[:], d["csbs_d"][:])
        nc.sync.dma_start(bdp[:], _r(d["bd_d"][:]))
        nc.sync.dma_start(bc2r[:], _r(d["bc2r_d"][:]))
        for e in range(EK):
            nc.sync.dma_start(xt[e][:, 768:NK], d["xt_d"][128 * e:128 * e + 128, 768:NK])
        for e in range(EK):
            nc.sync.dma_start(wq[e][:], d["wq_d"][128 * e:128 * e + 128, :])
        for e in range(EK):
            nc.sync.dma_start(wv[e][:], d["wv_d"][128 * e:128 * e + 128, :])
        vets = []
        for j in range(NJT):
            vet = vep.tile([128, NKV * HD], bf16, tag="vet", name=f"vet{j}")
            nc.sync.dma_start(vet[:], d["ve_d"][128 * j:128 * j + 128, :])
            vets.append(vet)
        nc.sync.dma_start(negi[:], d["negi_d"][:])
        nc.sync.dma_start(exlo[:], d["exlo_d"][:])
        nc.sync.dma_start(exup[:], d["exup_d"][:])
        nc.sync.dma_start(on4[:], d["on4_d"][:])
        for e in range(EK):
            nc.sync.dma_start(wp[e][:], d["wp_d"][128 * e:128 * e + 128, :])

        # ---- rope machinery: batched Newton rsqrt, pools passed per phase ----
        class Rope:
            def __init__(self, psKQ, psRQ, psB, act_heavy):
                self.psKQ, self.psRQ, self.psB = psKQ, psRQ, psB
                self.act_heavy = act_heavy
                self.batch, self.pqb = [], None

            def newton_step(self, y, pq_, dst):
                nr = pq_.shape[0]
                t = rsq.tile([128, 512], f32, tag="nt", name="nt")
                nc.vector.tensor_mul(t[0:nr, :], y, y)
                nc.vector.tensor_mul(t[0:nr, :], t[0:nr, :], pq_)
                nc.vector.tensor_scalar(t[0:nr, :], t[0:nr, :], -0.5, 1.5, OP.mult, OP.add)
                nc.vector.tensor_mul(dst, y, t[0:nr, :])

            def flush(self):
                if not self.batch:
                    return
                pqb = self.pqb
                nr = 32 * (len(self.batch) - 1) + 2
                pq_ = pqb[0:nr, :]
                ii = rsq.tile([128, 512], i32, tag="ii", name="ii")
                nc.vector.tensor_scalar(ii[0:nr, :], pq_.bitcast(i32), 1, 0,
                                        OP.logical_shift_right)
                nc.vector.tensor_scalar(ii[0:nr, :], ii[0:nr, :], -1, MAGIC0,
                                        OP.mult, OP.add)
                y0 = ii[0:nr, :].bitcast(f32)
                rcp = rsq.tile([128, 512], f32r, tag="rcp", name="rcp")
                with nc.allow_low_precision(reason="rsqrt scale in f32r for matmul bcast"):
                    self.newton_step(y0, pq_, rcp[0:nr, :])
                for (s, rot, w, outs) in self.batch:
                    prq = self.psRQ.tile([128, w], f32, tag="prq", name="prq")
                    nc.tensor.matmul(prq[:], bc2r[32 * s:32 * s + 2, :],
                                     rcp[32 * s:32 * s + 2, 0:w],
                                     start=True, stop=True, tile_position=(32 * s, 0))
                    nc.vector.tensor_mul(outs, rot[:], prq[:])
                self.batch = []
                self.pqb = None

            BDP_OFF = [0, 98, 196, 294]

            def up(self, pr, c0, w, outs, nb=4, last=False):
                if self.pqb is None:
                    self.pqb = self.psB.tile([128, 512], f32, tag="pqb", name="pqb")
                s = len(self.batch)
                rot = rotp.tile([128, w], bf16, tag="rot", name="rot")
                # spread across ACT/Pool/DVE (pre-attention phase: ACT idle)
                prC = tmpA.tile([128, w], bf16, tag="prc", name="prc")
                nc.scalar.copy(prC[:], pr[:])
                prS = tmpA.tile([128, w], bf16, tag="prs", name="prs")
                nc.vector.stream_shuffle(prS[:], prC[:], SWAP_MASK)
                ta = tmpA.tile([128, w], bf16, tag="ta", name="ta")
                nc.gpsimd.tensor_mul(ta[:], prC[:], csa[:, c0:c0 + w])
                tbs = tmpA.tile([128, w], bf16, tag="tbs", name="tbs")
                nc.gpsimd.tensor_mul(tbs[:], prS[:], csbs[:, c0:c0 + w])
                nc.gpsimd.tensor_add(rot[:], ta[:], tbs[:])
                sq = tmpA.tile([128, w], f32r, tag="sq", name="sq")
                nc.scalar.activation(sq[:], rot[:], AF.Square)
                off = self.BDP_OFF[s]
                nr = 32 * (nb - 1) + 2
                nc.tensor.matmul(self.pqb[0:nr, 0:w],
                                 bdp[:, off:off + nr], sq[:],
                                 start=(s == 0), stop=last)
                self.batch.append((s, rot, w, outs))
                if last:
                    self.flush()

        def qproj(psKQ, p):
            pr = psKQ.tile([128, CH], f32, tag="pk", name=f"pa{p}")
            for e in range(EK):
                nc.tensor.matmul(pr[:], wq[e][:, 128 * p:128 * p + 128],
                                 xt[e][:, 0:CH], start=(e == 0), stop=(e == EK - 1))
            return pr

        def kproj(psKQ, t, c):
            c0 = 512 * c
            pr = psKQ.tile([128, 512], f32, tag="pk", name="pk")
            for e in range(EK):
                nc.tensor.matmul(pr[:], wk[e][:, 128 * t:128 * t + 128],
                                 xt[e][:, c0:c0 + 512],
                                 start=(e == 0), stop=(e == EK - 1))
            return pr

        def attn_pair(p, psS, psO, ptp, tmpB):
            hA, hB = PAIRS[p]
            ktt = kt[hA // 8]
            ots = []
            for idx, h in enumerate((hA, hB)):
                g = h // 4
                ot = psO.tile([HD + 1, CH], f32, tag="ot", name=f"ot{h}")
                vg0 = vx[0][:, (HD + 1) * g:(HD + 1) * g + HD + 1]
                nc.tensor.matmul(ot[:], vg0, zP[:], start=True, stop=False)
                ots.append(ot)
            for jt in range(NJT):
                il0 = max(0, jt - 8)
                il1 = min(3, jt)
                iw0 = 128 * il0
                w = 128 * (il1 - il0 + 1)
                blo = jt <= 3
                bup = jt >= 8
                s2 = psS.tile([128, 1024], f32, tag="st", name="st")
                nc.tensor.matmul(s2[:, 0:w], ktt[0:64, 128 * jt:128 * jt + 128],
                                 qt[p][0:64, iw0:iw0 + w],
                                 start=True, stop=not (blo or bup))
                nc.tensor.matmul(s2[:, 512:512 + w], ktt[64:128, 128 * jt:128 * jt + 128],
                                 qt[p][64:128, iw0:iw0 + w],
                                 start=True, stop=not (blo or bup))
                if blo:
                    nc.tensor.matmul(s2[:, w - 128:w], negi[:], exlo[:],
                                     start=False, stop=True)
                    nc.tensor.matmul(s2[:, 512 + w - 128:512 + w], negi[:], exlo[:],
                                     start=False, stop=True)
                if bup:
                    nc.tensor.matmul(s2[:, 0:128], negi[:], exup[:],
                                     start=False, stop=True)
                    nc.tensor.matmul(s2[:, 512:512 + 128], negi[:], exup[:],
                                     start=False, stop=True)
                pt = ptp.tile([128, 1024], bf16, tag="pt", name="pt")
                sv = s2[:].rearrange("q (b c) -> q b c", b=2)[:, :, 0:w]
                pv_ = pt[:].rearrange("q (b c) -> q b c", b=2)[:, :, 0:w]
                nc.scalar.activation(pv_, sv, AF.Exp)
                for idx, h in enumerate((hA, hB)):
                    off = 512 * idx
                    g = h // 4
                    vsl = vx[jt][:, (HD + 1) * g:(HD + 1) * g + HD + 1]
                    nc.tensor.matmul(ots[idx][:, iw0:iw0 + w], vsl, pt[:, off:off + w],
                                     start=False, stop=(jt == NJT - 1))
            for idx, h in enumerate((hA, hB)):
                ot = ots[idx]
                rs = tmpB.tile([1, CH], f32, tag="rs", name=f"rs{h}")
                nc.vector.reciprocal(rs[:], ot[HD:HD + 1, :])
                rsb = tmpB.tile([64, CH], f32, tag="rsb", name=f"rsb{h}")
                nc.gpsimd.partition_broadcast(rsb[:], rs[:])
                nc.vector.tensor_mul(yt[h // 2][64 * (h % 2):64 * (h % 2) + 64, :],
                                     ot[0:HD, :], rsb[:])

        # ========== phase A: gates, K, V, Q projections + ropes ==========
        with (
            tc.tile_pool(name="psPR", bufs=2, space="PSUM") as psPR,
            tc.tile_pool(name="psRQ", bufs=2, space="PSUM") as psRQ,
            tc.tile_pool(name="psV", bufs=2, space="PSUM") as psV,
            tc.tile_pool(name="psB", bufs=2, space="PSUM") as psB,
        ):
            ropeA = Rope(psPR, psRQ, psB, act_heavy=True)

            gates = []
            for j in range(NJT):
                pg = psV.tile([128, NKV * HD], f32, tag="pv", name=f"pg{j}")
                nc.tensor.matmul(pg[:, 0:NKV], xt[0][0:32, 128 * j:128 * j + 128],
                                 wg[:], start=True, stop=True)
                gt = xa.tile([128, NKV], f32, tag=f"gate{j}", name=f"gate{j}")
                nc.scalar.activation(gt[:], pg[:, 0:NKV], AF.Tanh, scale=0.5)
                g2 = xa.tile([128, NKV], bf16, tag=f"gate2_{j}", name=f"gate2_{j}")
                nc.vector.tensor_scalar_add(g2[:], gt[:], 1.0)
                gates.append(g2)

            for t in range(2):
                for c in range(3):
                    pr = kproj(psPR, t, c)
                    c0 = 512 * c
                    ropeA.up(pr, c0, 512, kt[t][:, c0:c0 + 512], nb=3, last=(c == 2))

            for p in range(8):
                pr = qproj(psPR, p)
                ropeA.up(pr, 0, CH, qt[p][:], nb=4, last=(p in (3, 7)))

            for j in range(NJT):
                pv = psV.tile([128, NKV * HD], f32, tag="pv", name="pv")
                for e in range(EK):
                    nc.tensor.matmul(pv[:], xt[e][:, 128 * j:128 * j + 128],
                                     wv[e][:], start=(e == 0), stop=(e == EK - 1))
                vet = vets[j]
                vg = vep.tile([128, NKV * HD], bf16, tag="vg", name="vg", bufs=2)
                nc.gpsimd.tensor_mul(
                    vg[:].rearrange("p (g c) -> p g c", c=HD),
                    vet[:].rearrange("p (g c) -> p g c", c=HD),
                    gates[j][:].unsqueeze(2).broadcast_to([128, NKV, HD]))
                vxv = vx[j][:].rearrange("p (g c) -> p g c", c=HD + 1)
                nc.vector.tensor_add(
                    vxv[:, :, 0:HD],
                    vg[:].rearrange("p (g c) -> p g c", c=HD),
                    pv[:].rearrange("p (g c) -> p g c", c=HD))
                nc.vector.tensor_copy(vxv[:, :, HD], on4[:, NKV * j:NKV * j + NKV])


        # ========== phase B: attention ==========
        with (
            tc.tile_pool(name="tmpB", bufs=2) as tmpB,
            tc.tile_pool(name="ptp", bufs=3) as ptp,
            tc.tile_pool(name="psS", bufs=2, space="PSUM") as psS,
            tc.tile_pool(name="psO", bufs=4, space="PSUM") as psO,
        ):
            for p in range(8):
                attn_pair(p, psS, psO, ptp, tmpB)

        # ========== phase C: output projection ==========
        with (
            tc.tile_pool(name="pop", bufs=2) as pop,
            tc.tile_pool(name="psP", bufs=2, space="PSUM") as psP,
        ):
            for it in range(4):
                for half in range(2):
                    pp = psP.tile([128, 512], f32, tag="pp", name="pp")
                    for f in range(EK):
                        nc.tensor.matmul(pp[:], yt[f][:, 128 * it:128 * it + 128],
                                         wp[f][:, 512 * half:512 * half + 512],
                                         start=(f == 0), stop=(f == EK - 1))
                    po = pop.tile([128, 512], f32, tag="po", name="po")
                    nc.vector.tensor_copy(po[:], pp[:])
                    nc.sync.dma_start(
                        d["out_d"][128 * it:128 * it + 128,
                                   512 * half:512 * half + 512],
                        po[:])



# ---------------- host prep ----------------

def host_prep(inputs):
    bfd = ml_dtypes.bfloat16
    x = np.asarray(inputs["x"], np.float32)
    ve = np.asarray(inputs["ve"], np.float32)
    cos = np.asarray(inputs["cos"], np.float32)
    sin = np.asarray(inputs["sin"], np.float32)
    wq = np.asarray(inputs["wq"], np.float32)
    wk = np.asarray(inputs["wk"], np.float32)
    wv = np.asarray(inputs["wv"], np.float32)
    wproj = np.asarray(inputs["wproj"], np.float32)
    wgate = np.asarray(inputs["wgate"], np.float32)

    def rope_perm(nh):
        idx = np.empty(nh * 64, np.int64)
        for h in range(nh):
            for dd in range(32):
                for half in range(2):
                    idx[h * 64 + 2 * dd + half] = h * 64 + 32 * half + dd
        return idx

    XT = np.zeros((B, NE, TPAD), bfd)
    XT[:, :, :T] = x.transpose(0, 2, 1).astype(bfd)
    VEP = np.zeros((B, TPAD, NKV * HD), bfd)
    VEP[:, :T] = ve.astype(bfd)

    wq_perm = wq.T[:, rope_perm(NH)]
    cols = []
    for hA, hB in PAIRS:
        cols.extend(range(64 * hA, 64 * hA + 64))
        cols.extend(range(64 * hB, 64 * hB + 64))
    wq_t = np.ascontiguousarray(wq_perm[:, cols]).astype(bfd)
    wk_t = np.ascontiguousarray(wk.T[:, rope_perm(NKV)]).astype(bfd)
    wv_t = np.ascontiguousarray(wv.T).astype(bfd)
    wp_t = np.ascontiguousarray(wproj.T).astype(bfd)
    wg_t = np.ascontiguousarray(wgate.T).astype(bfd)

    cosT = np.zeros((32, TPAD), np.float32)
    sinT = np.zeros((32, TPAD), np.float32)
    cosT[:, :T] = cos[0, :, 0, :].T
    sinT[:, :T] = sin[0, :, 0, :].T
    csa64 = np.empty((64, TPAD), np.float32)
    csb64 = np.empty((64, TPAD), np.float32)
    csa64[0::2] = cosT
    csa64[1::2] = cosT
    csb64[0::2] = -sinT
    csb64[1::2] = sinT
    CSA = np.concatenate([csa64, csa64], 0)
    CSB = np.concatenate([csb64, csb64], 0)
    perm = np.arange(128) ^ 1
    CSBS = CSB[perm]  # pre-shuffled: csbs[p] = csb[p^1]

    ones4 = np.zeros((TPAD, NKV), bfd)
    ones4[:T] = 1.0

    jj = np.arange(128)[:, None]   # key (partition)
    ii = np.arange(128)[None, :]   # query (column)
    exlo = (ii > jj).astype(bfd)   # excluded at diagonal block: q > k
    exup = (ii < jj).astype(bfd)   # excluded at window block: q < k
    negi = (-60.0 * np.eye(128)).astype(bfd)
    # per-slot sum-of-squares selectors, zero-padded so the matmul dst stays
    # at partition 0: slot s uses cols off_s .. off_s+32s+2, nonzero only in
    # the last two columns (strip selectors scaled by 1/8).
    bdp = np.zeros((128, 392), np.float32)
    for s in range(4):
        bdp[:64, 98 * s + 32 * s] = 0.125
        bdp[64:, 98 * s + 32 * s + 1] = 0.125
    bc2r = np.zeros((128, 128), np.float32)
    for s in range(4):
        bc2r[32 * s, :64] = 1.0
        bc2r[32 * s + 1, 64:] = 1.0

    in_maps = []
    for c in range(8):
        b, ci = c // 4, c % 4
        q0 = CH * ci
        o4 = ones4[q0:q0 + NK]  # [1536, 4]
        o4p = np.ascontiguousarray(o4.reshape(NJT, 128, NKV).transpose(1, 0, 2).reshape(128, NJT * NKV))
        in_maps.append({
            "xt": np.ascontiguousarray(XT[b][:, q0:q0 + NK]),
            "wqt": wq_t, "wkt": wk_t, "wvt": wv_t, "wgt": wg_t, "wpt": wp_t,
            "csa": np.ascontiguousarray(CSA[:, q0:q0 + NK].astype(bfd)),
            "csbs": np.ascontiguousarray(CSBS[:, q0:q0 + NK].astype(bfd)),
            "ve": np.ascontiguousarray(VEP[b][q0:q0 + NK]),
            "ones4": o4p,
            "exlo": exlo, "exup": exup, "negi": negi,
            "bdp": bdp, "bc2r": bc2r,
        })
    return in_maps


def kernel(**inputs):
    global _COMPILED
    if _COMPILED is None:
        _COMPILED = build_program()
    nc = _COMPILED
    in_maps = host_prep(inputs)

    from concourse.bass_utils import run_bass_kernel_spmd
    res = run_bass_kernel_spmd(nc, in_maps, list(range(8)))

    out = np.empty((B, T, NE), np.float32)
    for c in range(8):
        b, ci = c // 4, c % 4
        out[b, CH * ci:CH * ci + CH] = res.results[c]["out"]
    return out



# revision 4
# speedup vs baseline: 2.9995x; 2.9995x over previous
"""Trainium2 Bass kernel for nn_CausalSelfAttention_57861799412149.

Self-contained: takes FULL inputs (as in reference.setup_inputs()), returns the
FULL output. Sharding: sequence-parallel — 8 cores = 2 batches x 4 contiguous
query chunks of 512; each core computes K/V only for its 1536-key window
(query chunk + 1024 lookahead, zero-padded past T) and writes an exact
[512, 1024] slice of the output. No collectives.

v3 notes:
- All big matmuls in bf16 (inputs cast on host); PSUM accumulation in f32.
- Sliding-window masks folded into the S^T matmul accumulation as
  (-60*I) x excl-triangle bf16 matmuls — no post-exp masking ops.
- rmsnorm 1/sqrt via DVE Newton iteration (bit-trick seed, 2 steps), batched
  4 ropes per PSUM tile at 32-aligned partition slots. No ACT Sqrt at all, so
  the single ACT table set (exp/tanh/square/copy) loads once and attention
  never waits on table switches or rope grouping.
- Rope chain spread across engines: ACT copy (PSUM->SBUF) + Square, Pool
  muls with host-preshuffled cos/sin, DVE shuffle/add; per-token scales are
  broadcast across partitions with a bc2-style matmul.
- Softmax has no max-subtraction (|s| <= 8); denominators come free from a
  ones-column appended to V; normalization applied to O^T.
"""
import sys

sys.path.insert(0, "/opt/trn_rl_repo")

import numpy as np
import ml_dtypes

import concourse.bass as bass
import concourse.tile as tile
from concourse import bacc, mybir

B, T, NE = 2, 2048, 1024
NH, NKV, HD = 16, 4, 64
CH = 512            # queries per core
NK = 1536           # key window per core (padded)
TPAD = 3072
EK = NE // 128      # 8 contraction tiles
NJT = NK // 128     # 12 key tiles

f32 = mybir.dt.float32
f32r = mybir.dt.float32r
bf16 = mybir.dt.bfloat16
i32 = mybir.dt.int32
AF = mybir.ActivationFunctionType
OP = mybir.AluOpType
SWAP_MASK = [m for i in range(0, 32, 2) for m in (i + 1, i)]
MAGIC0 = 0x5F3759DF  # rsqrt bit-trick seed magic: y0 = bitcast(MAGIC - (i >> 1))
# head pairs sharing one [128, .] tile: strips (0, 64). Pair (h, h+4) keeps the
# kv-group parity aligned with the kt pair tiles so matmul base partitions match.
PAIRS = [(0, 4), (1, 5), (2, 6), (3, 7), (8, 12), (9, 13), (10, 14), (11, 15)]

_COMPILED = None


def _r(ap):
    return ap.bitcast(f32r)


def build_program(repeat=1, unroll=False):
    nc = bacc.Bacc("TRN2", target_bir_lowering=False, debug=False, num_devices=8)

    def din(name, shape, dt=bf16):
        return nc.dram_tensor(name, shape, dt, kind="ExternalInput").ap()

    xt_d = din("xt", [NE, NK])
    wq_d = din("wqt", [NE, NE])
    wk_d = din("wkt", [NE, NKV * HD])
    wv_d = din("wvt", [NE, NKV * HD])
    wg_d = din("wgt", [32, NKV])
    wp_d = din("wpt", [NE, NE])
    csa_d = din("csa", [128, NK])
    csbs_d = din("csbs", [128, NK])
    ve_d = din("ve", [NK, NKV * HD])
    on4_d = din("ones4", [128, NJT * NKV])
    exlo_d = din("exlo", [128, 128])
    exup_d = din("exup", [128, 128])
    negi_d = din("negi", [128, 128])
    bd_d = din("bdp", [128, 392], f32)
    bc2r_d = din("bc2r", [128, 128], f32)
    out_d = nc.dram_tensor("out", [CH, NE], f32, kind="ExternalOutput").ap()

    ctx_vars = locals()
    with tile.TileContext(nc) as tc:
        if repeat == 1:
            _build(nc, tc, ctx_vars)
        elif unroll:
            for _ in range(repeat):
                _build(nc, tc, ctx_vars)
        else:
            with tc.For_i(0, repeat,
                          hint_engines=(mybir.EngineType.PE,
                                        mybir.EngineType.DVE,
                                        mybir.EngineType.Activation)):
                _build(nc, tc, ctx_vars)

    nc.compile()
    return nc


def _build(nc, tc, d):
    from contextlib import ExitStack

    ctx = ExitStack()
    with ctx:
        # ---------------- persistent pools (live whole kernel) ----------------
        consts = ctx.enter_context(tc.tile_pool(name="consts", bufs=1))
        qtp = ctx.enter_context(tc.tile_pool(name="qtp", bufs=1))
        ktp = ctx.enter_context(tc.tile_pool(name="ktp", bufs=1))
        vxp = ctx.enter_context(tc.tile_pool(name="vxp", bufs=1))
        ytp = ctx.enter_context(tc.tile_pool(name="ytp", bufs=1))
        wqp = ctx.enter_context(tc.tile_pool(name="wqp", bufs=1))
        wpp = ctx.enter_context(tc.tile_pool(name="wpp", bufs=1))
        xa = ctx.enter_context(tc.tile_pool(name="xa", bufs=1))
        vep = ctx.enter_context(tc.tile_pool(name="vep", bufs=12))
        tmpA = ctx.enter_context(tc.tile_pool(name="tmpA", bufs=3))
        rotp = ctx.enter_context(tc.tile_pool(name="rotp", bufs=6))
        rsq = ctx.enter_context(tc.tile_pool(name="rsq", bufs=2))

        bdp = consts.tile([128, 392], f32r, tag="bdp")
        bc2r = consts.tile([128, 128], f32r, tag="bc2r")
        negi = consts.tile([128, 128], bf16, tag="negi")
        exlo = consts.tile([128, 128], bf16, tag="exlo")
        exup = consts.tile([128, 128], bf16, tag="exup")
        on4 = consts.tile([128, NJT * NKV], bf16, tag="on4")
        zP = consts.tile([128, CH], bf16, tag="zP")
        nc.vector.memset(zP[:], 0.0)
        csa = consts.tile([128, NK], bf16, tag="csa")
        csbs = consts.tile([128, NK], bf16, tag="csbs")

        qt = [qtp.tile([128, CH], bf16, tag=f"qt{p}", name=f"qt{p}") for p in range(8)]
        kt = [ktp.tile([128, NK], bf16, tag=f"kt{t}", name=f"kt{t}") for t in range(2)]
        vx = [vxp.tile([128, NKV * (HD + 1)], bf16, tag=f"vx{j}", name=f"vx{j}") for j in range(NJT)]
        yt = [ytp.tile([128, CH], bf16, tag=f"yt{f}", name=f"yt{f}") for f in range(EK)]
        wq = [wqp.tile([128, NE], bf16, tag=f"wq{e}", name=f"wq{e}") for e in range(EK)]
        wp = [wpp.tile([128, NE], bf16, tag=f"wp{e}", name=f"wp{e}") for e in range(EK)]

        wg = xa.tile([32, NKV], bf16, tag="wg")
        nc.sync.dma_start(wg[:], d["wg_d"][:])
        xt = [xa.tile([128, NK], bf16, tag=f"xt{e}", name=f"xt{e}") for e in range(EK)]
        wk = [xa.tile([128, NKV * HD], bf16, tag=f"wk{e}", name=f"wk{e}") for e in range(EK)]
        wv = [xa.tile([128, NKV * HD], bf16, tag=f"wv{e}", name=f"wv{e}") for e in range(EK)]
        for e in range(EK):
            nc.sync.dma_start(xt[e][:, 0:768], d["xt_d"][128 * e:128 * e + 128, 0:768])
            nc.sync.dma_start(wk[e][:], d["wk_d"][128 * e:128 * e + 128, :])
        nc.sync.dma_start(csa[:], d["csa_d"][:])
        nc.sync.dma_start(csbs[:], d["csbs_d"][:])
        nc.sync.dma_start(bdp[:], _r(d["bd_d"][:]))
        nc.sync.dma_start(bc2r[:], _r(d["bc2r_d"][:]))
        for e in range(EK):
            nc.sync.dma_start(xt[e][:, 768:NK], d["xt_d"][128 * e:128 * e + 128, 768:NK])
        for e in range(EK):
            nc.sync.dma_start(wq[e][:], d["wq_d"][128 * e:128 * e + 128, :])
        for e in range(EK):
            nc.sync.dma_start(wv[e][:], d["wv_d"][128 * e:128 * e + 128, :])
        vets = []
        for j in range(NJT):
            vet = vep.tile([128, NKV * HD], bf16, tag="vet", name=f"vet{j}")
            nc.sync.dma_start(vet[:], d["ve_d"][128 * j:128 * j + 128, :])
            vets.append(vet)
        nc.sync.dma_start(negi[:], d["negi_d"][:])
        nc.sync.dma_start(exlo[:], d["exlo_d"][:])
        nc.sync.dma_start(exup[:], d["exup_d"][:])
        nc.sync.dma_start(on4[:], d["on4_d"][:])
        for e in range(EK):
            nc.sync.dma_start(wp[e][:], d["wp_d"][128 * e:128 * e + 128, :])

        # ---- rope machinery: batched Newton rsqrt, pools passed per phase ----
        class Rope:
            def __init__(self, psKQ, psRQ, psB, act_heavy):
                self.psKQ, self.psRQ, self.psB = psKQ, psRQ, psB
                self.act_heavy = act_heavy
                self.batch, self.pqb = [], None

            def newton_step(self, y, pq_, dst):
                nr = pq_.shape[0]
                t = rsq.tile([128, 512], f32, tag="nt", name="nt")
                nc.vector.tensor_mul(t[0:nr, :], y, y)
                nc.vector.tensor_mul(t[0:nr, :], t[0:nr, :], pq_)
                nc.vector.tensor_scalar(t[0:nr, :], t[0:nr, :], -0.5, 1.5, OP.mult, OP.add)
                nc.vector.tensor_mul(dst, y, t[0:nr, :])

            def flush(self):
                if not self.batch:
                    return
                pqb = self.pqb
                nr = 32 * (len(self.batch) - 1) + 2
                pq_ = pqb[0:nr, :]
                ii = rsq.tile([128, 512], i32, tag="ii", name="ii")
                nc.vector.tensor_scalar(ii[0:nr, :], pq_.bitcast(i32), 1, 0,
                                        OP.logical_shift_right)
                nc.vector.tensor_scalar(ii[0:nr, :], ii[0:nr, :], -1, MAGIC0,
                                        OP.mult, OP.add)
                y0 = ii[0:nr, :].bitcast(f32)
                rcp = rsq.tile([128, 512], f32r, tag="rcp", name="rcp")
                with nc.allow_low_precision(reason="rsqrt scale in f32r for matmul bcast"):
                    self.newton_step(y0, pq_, rcp[0:nr, :])
                for (s, rot, w, outs) in self.batch:
                    prq = self.psRQ.tile([128, w], f32, tag="prq", name="prq")
                    nc.tensor.matmul(prq[:], bc2r[32 * s:32 * s + 2, :],
                                     rcp[32 * s:32 * s + 2, 0:w],
                                     start=True, stop=True, tile_position=(32 * s, 0))
                    nc.vector.tensor_mul(outs, rot[:], prq[:])
                self.batch = []
                self.pqb = None

            BDP_OFF = [0, 98, 196, 294]

            def up(self, pr, c0, w, outs, nb=4, last=False):
                if self.pqb is None:
                    self.pqb = self.psB.tile([128, 512], f32, tag="pqb", name="pqb")
                s = len(self.batch)
                rot = rotp.tile([128, w], bf16, tag="rot", name="rot")
                # spread across ACT/Pool/DVE (pre-attention phase: ACT idle)
                prC = tmpA.tile([128, w], bf16, tag="prc", name="prc")
                nc.scalar.copy(prC[:], pr[:])
                prS = tmpA.tile([128, w], bf16, tag="prs", name="prs")
                nc.vector.stream_shuffle(prS[:], prC[:], SWAP_MASK)
                ta = tmpA.tile([128, w], bf16, tag="ta", name="ta")
                nc.vector.tensor_mul(ta[:], prC[:], csa[:, c0:c0 + w])
                tbs = tmpA.tile([128, w], bf16, tag="tbs", name="tbs")
                nc.gpsimd.tensor_mul(tbs[:], prS[:], csbs[:, c0:c0 + w])
                nc.vector.tensor_add(rot[:], ta[:], tbs[:])
                sq = tmpA.tile([128, w], f32r, tag="sq", name="sq")
                nc.scalar.activation(sq[:], rot[:], AF.Square)
                off = self.BDP_OFF[s]
                nr = 32 * (nb - 1) + 2
                nc.tensor.matmul(self.pqb[0:nr, 0:w],
                                 bdp[:, off:off + nr], sq[:],
                                 start=(s == 0), stop=last)
                self.batch.append((s, rot, w, outs))
                if last:
                    self.flush()

        def qproj(psKQ, p):
            pr = psKQ.tile([128, CH], f32, tag="pk", name=f"pa{p}")
            for e in range(EK):
                nc.tensor.matmul(pr[:], wq[e][:, 128 * p:128 * p + 128],
                                 xt[e][:, 0:CH], start=(e == 0), stop=(e == EK - 1))
            return pr

        def kproj(psKQ, t, c):
            c0 = 512 * c
            pr = psKQ.tile([128, 512], f32, tag="pk", name="pk")
            for e in range(EK):
                nc.tensor.matmul(pr[:], wk[e][:, 128 * t:128 * t + 128],
                                 xt[e][:, c0:c0 + 512],
                                 start=(e == 0), stop=(e == EK - 1))
            return pr

        def attn_pair(p, psS, psO, ptp, tmpB):
            hA, hB = PAIRS[p]
            ktt = kt[hA // 8]
            ots = []
            for idx, h in enumerate((hA, hB)):
                g = h // 4
                ot = psO.tile([HD + 1, CH], f32, tag="ot", name=f"ot{h}")
                vg0 = vx[0][:, (HD + 1) * g:(HD + 1) * g + HD + 1]
                nc.tensor.matmul(ot[:], vg0, zP[:], start=True, stop=False)
                ots.append(ot)
            for jt in range(NJT):
                il0 = max(0, jt - 8)
                il1 = min(3, jt)
                iw0 = 128 * il0
                w = 128 * (il1 - il0 + 1)
                blo = jt <= 3
                bup = jt >= 8
                s2 = psS.tile([128, 1024], f32, tag="st", name="st")
                nc.tensor.matmul(s2[:, 0:w], ktt[0:64, 128 * jt:128 * jt + 128],
                                 qt[p][0:64, iw0:iw0 + w],
                                 start=True, stop=not (blo or bup))
                nc.tensor.matmul(s2[:, 512:512 + w], ktt[64:128, 128 * jt:128 * jt + 128],
                                 qt[p][64:128, iw0:iw0 + w],
                                 start=True, stop=not (blo or bup))
                if blo:
                    nc.tensor.matmul(s2[:, w - 128:w], negi[:], exlo[:],
                                     start=False, stop=True)
                    nc.tensor.matmul(s2[:, 512 + w - 128:512 + w], negi[:], exlo[:],
                                     start=False, stop=True)
                if bup:
                    nc.tensor.matmul(s2[:, 0:128], negi[:], exup[:],
                                     start=False, stop=True)
                    nc.tensor.matmul(s2[:, 512:512 + 128], negi[:], exup[:],
                                     start=False, stop=True)
                pt = ptp.tile([128, 1024], bf16, tag="pt", name="pt")
                sv = s2[:].rearrange("q (b c) -> q b c", b=2)[:, :, 0:w]
                pv_ = pt[:].rearrange("q (b c) -> q b c", b=2)[:, :, 0:w]
                nc.scalar.activation(pv_, sv, AF.Exp)
                for idx, h in enumerate((hA, hB)):
                    off = 512 * idx
                    g = h // 4
                    vsl = vx[jt][:, (HD + 1) * g:(HD + 1) * g + HD + 1]
                    nc.tensor.matmul(ots[idx][:, iw0:iw0 + w], vsl, pt[:, off:off + w],
                                     start=False, stop=(jt == NJT - 1))
            for idx, h in enumerate((hA, hB)):
                ot = ots[idx]
                ds = tmpB.tile([1, CH], f32, tag="ds", name=f"ds{h}")
                nc.vector.tensor_copy(ds[:], ot[HD:HD + 1, :])
                rs = tmpB.tile([1, CH], f32, tag="rs", name=f"rs{h}")
                nc.vector.reciprocal_approx_fast(rs[:], ds[:])
                rsb = tmpB.tile([64, CH], f32, tag="rsb", name=f"rsb{h}")
                nc.gpsimd.partition_broadcast(rsb[:], rs[:])
                nc.vector.tensor_mul(yt[h // 2][64 * (h % 2):64 * (h % 2) + 64, :],
                                     ot[0:HD, :], rsb[:])

        # ========== phase A: gates, K, V, Q projections + ropes ==========
        with (
            tc.tile_pool(name="psPR", bufs=2, space="PSUM") as psPR,
            tc.tile_pool(name="psRQ", bufs=2, space="PSUM") as psRQ,
            tc.tile_pool(name="psV", bufs=2, space="PSUM") as psV,
            tc.tile_pool(name="psB", bufs=2, space="PSUM") as psB,
        ):
            ropeA = Rope(psPR, psRQ, psB, act_heavy=True)

            gates = []
            for j in range(NJT):
                pg = psV.tile([128, NKV * HD], f32, tag="pv", name=f"pg{j}")
                nc.tensor.matmul(pg[:, 0:NKV], xt[0][0:32, 128 * j:128 * j + 128],
                                 wg[:], start=True, stop=True)
                gt = xa.tile([128, NKV], f32, tag=f"gate{j}", name=f"gate{j}")
                nc.scalar.activation(gt[:], pg[:, 0:NKV], AF.Tanh, scale=0.5)
                g2 = xa.tile([128, NKV], bf16, tag=f"gate2_{j}", name=f"gate2_{j}")
                nc.vector.tensor_scalar_add(g2[:], gt[:], 1.0)
                gates.append(g2)

            for t in range(2):
                for c in range(3):
                    pr = kproj(psPR, t, c)
                    c0 = 512 * c
                    ropeA.up(pr, c0, 512, kt[t][:, c0:c0 + 512], nb=3, last=(c == 2))

            for p in range(8):
                pr = qproj(psPR, p)
                ropeA.up(pr, 0, CH, qt[p][:], nb=4, last=(p in (3, 7)))

            for j in range(NJT):
                pv = psV.tile([128, NKV * HD], f32, tag="pv", name="pv")
                for e in range(EK):
                    nc.tensor.matmul(pv[:], xt[e][:, 128 * j:128 * j + 128],
                                     wv[e][:], start=(e == 0), stop=(e == EK - 1))
                vet = vets[j]
                vg = vep.tile([128, NKV * HD], bf16, tag="vg", name="vg", bufs=2)
                nc.gpsimd.tensor_mul(
                    vg[:].rearrange("p (g c) -> p g c", c=HD),
                    vet[:].rearrange("p (g c) -> p g c", c=HD),
                    gates[j][:].unsqueeze(2).broadcast_to([128, NKV, HD]))
                vxv = vx[j][:].rearrange("p (g c) -> p g c", c=HD + 1)
                nc.vector.tensor_add(
                    vxv[:, :, 0:HD],
                    vg[:].rearrange("p (g c) -> p g c", c=HD),
                    pv[:].rearrange("p (g c) -> p g c", c=HD))
                nc.vector.tensor_copy(vxv[:, :, HD], on4[:, NKV * j:NKV * j + NKV])


        # ========== phase B: attention ==========
        with (
            tc.tile_pool(name="tmpB", bufs=2) as tmpB,
            tc.tile_pool(name="ptp", bufs=3) as ptp,
            tc.tile_pool(name="psS", bufs=2, space="PSUM") as psS,
            tc.tile_pool(name="psO", bufs=4, space="PSUM") as psO,
        ):
            for p in range(8):
                attn_pair(p, psS, psO, ptp, tmpB)

        # ========== phase C: output projection ==========
        with (
            tc.tile_pool(name="pop", bufs=2) as pop,
            tc.tile_pool(name="psP", bufs=2, space="PSUM") as psP,
        ):
            for it in range(4):
                for half in range(2):
                    pp = psP.tile([128, 512], f32, tag="pp", name="pp")
                    for f in range(EK):
                        nc.tensor.matmul(pp[:], yt[f][:, 128 * it:128 * it + 128],
                                         wp[f][:, 512 * half:512 * half + 512],
                                         start=(f == 0), stop=(f == EK - 1))
                    po = pop.tile([128, 512], f32, tag="po", name="po")
                    nc.vector.tensor_copy(po[:], pp[:])
                    nc.sync.dma_start(
                        d["out_d"][128 * it:128 * it + 128,
                                   512 * half:512 * half + 512],
                        po[:])



# ---------------- host prep ----------------

def host_prep(inputs):
    bfd = ml_dtypes.bfloat16
    x = np.asarray(inputs["x"], np.float32)
    ve = np.asarray(inputs["ve"], np.float32)
    cos = np.asarray(inputs["cos"], np.float32)
    sin = np.asarray(inputs["sin"], np.float32)
    wq = np.asarray(inputs["wq"], np.float32)
    wk = np.asarray(inputs["wk"], np.float32)
    wv = np.asarray(inputs["wv"], np.float32)
    wproj = np.asarray(inputs["wproj"], np.float32)
    wgate = np.asarray(inputs["wgate"], np.float32)

    def rope_perm(nh):
        idx = np.empty(nh * 64, np.int64)
        for h in range(nh):
            for dd in range(32):
                for half in range(2):
                    idx[h * 64 + 2 * dd + half] = h * 64 + 32 * half + dd
        return idx

    XT = np.zeros((B, NE, TPAD), bfd)
    XT[:, :, :T] = x.transpose(0, 2, 1).astype(bfd)
    VEP = np.zeros((B, TPAD, NKV * HD), bfd)
    VEP[:, :T] = ve.astype(bfd)

    wq_perm = wq.T[:, rope_perm(NH)]
    cols = []
    for hA, hB in PAIRS:
        cols.extend(range(64 * hA, 64 * hA + 64))
        cols.extend(range(64 * hB, 64 * hB + 64))
    wq_t = np.ascontiguousarray(wq_perm[:, cols]).astype(bfd)
    wk_t = np.ascontiguousarray(wk.T[:, rope_perm(NKV)]).astype(bfd)
    wv_t = np.ascontiguousarray(wv.T).astype(bfd)
    wp_t = np.ascontiguousarray(wproj.T).astype(bfd)
    wg_t = np.ascontiguousarray(wgate.T).astype(bfd)

    cosT = np.zeros((32, TPAD), np.float32)
    sinT = np.zeros((32, TPAD), np.float32)
    cosT[:, :T] = cos[0, :, 0, :].T
    sinT[:, :T] = sin[0, :, 0, :].T
    csa64 = np.empty((64, TPAD), np.float32)
    csb64 = np.empty((64, TPAD), np.float32)
    csa64[0::2] = cosT
    csa64[1::2] = cosT
    csb64[0::2] = -sinT
    csb64[1::2] = sinT
    CSA = np.concatenate([csa64, csa64], 0)
    CSB = np.concatenate([csb64, csb64], 0)
    perm = np.arange(128) ^ 1
    CSBS = CSB[perm]  # pre-shuffled: csbs[p] = csb[p^1]

    ones4 = np.zeros((TPAD, NKV), bfd)
    ones4[:T] = 1.0

    jj = np.arange(128)[:, None]   # key (partition)
    ii = np.arange(128)[None, :]   # query (column)
    exlo = (ii > jj).astype(bfd)   # excluded at diagonal block: q > k
    exup = (ii < jj).astype(bfd)   # excluded at window block: q < k
    negi = (-60.0 * np.eye(128)).astype(bfd)
    # per-slot sum-of-squares selectors, zero-padded so the matmul dst stays
    # at partition 0: slot s uses cols off_s .. off_s+32s+2, nonzero only in
    # the last two columns (strip selectors scaled by 1/8).
    bdp = np.zeros((128, 392), np.float32)
    for s in range(4):
        bdp[:64, 98 * s + 32 * s] = 0.125
        bdp[64:, 98 * s + 32 * s + 1] = 0.125
    bc2r = np.zeros((128, 128), np.float32)
    for s in range(4):
        bc2r[32 * s, :64] = 1.0
        bc2r[32 * s + 1, 64:] = 1.0

    in_maps = []
    for c in range(8):
        b, ci = c // 4, c % 4
        q0 = CH * ci
        o4 = ones4[q0:q0 + NK]  # [1536, 4]
        o4p = np.ascontiguousarray(o4.reshape(NJT, 128, NKV).transpose(1, 0, 2).reshape(128, NJT * NKV))
        in_maps.append({
            "xt": np.ascontiguousarray(XT[b][:, q0:q0 + NK]),
            "wqt": wq_t, "wkt": wk_t, "wvt": wv_t, "wgt": wg_t, "wpt": wp_t,
            "csa": np.ascontiguousarray(CSA[:, q0:q0 + NK].astype(bfd)),
            "csbs": np.ascontiguousarray(CSBS[:, q0:q0 + NK].astype(bfd)),
            "ve": np.ascontiguousarray(VEP[b][q0:q0 + NK]),
            "ones4": o4p,
            "exlo": exlo, "exup": exup, "negi": negi,
            "bdp": bdp, "bc2r": bc2r,
        })
    return in_maps


def kernel(**inputs):
    global _COMPILED
    if _COMPILED is None:
        _COMPILED = build_program()
    nc = _COMPILED
    in_maps = host_prep(inputs)

    from concourse.bass_utils import run_bass_kernel_spmd
    res = run_bass_kernel_spmd(nc, in_maps, list(range(8)))

    out = np.empty((B, T, NE), np.float32)
    for c in range(8):
        b, ci = c // 4, c % 4
        out[b, CH * ci:CH * ci + CH] = res.results[c]["out"]
    return out

